# revision 1
# baseline (speedup 1.0000x reference)
import sys

sys.path.insert(0, "/opt/trn_rl_repo")

import numpy as np
import ml_dtypes

import concourse.bacc as bacc
import concourse.bass as bass
import concourse.mybir as mybir
import concourse.tile as tile
from concourse.bass_utils import run_bass_kernel_spmd

F32 = mybir.dt.float32
BF16 = mybir.dt.bfloat16
AF = mybir.ActivationFunctionType
ALU = mybir.AluOpType
AX = mybir.AxisListType

# Problem constants (hardcoded per harness contract).
B, C, H, W = 4, 64, 128, 128
COUT1 = 128
NT = 9          # 3x3 taps
NFF = 4         # factor*factor subpixels
NCORES = 8
HL = H // 2     # 64 coarse rows per core
NYB = 4         # y-blocks for the weighted sum
YB = HL // NYB  # 16 rows per block

_cached = {}


def ap_of(t, off, dims):
    base = t[:]
    return bass.AP(base.tensor, base.offset + off, dims)


def build_nc():
    nc = bacc.Bacc("TRN2", target_bir_lowering=False, debug=False, num_devices=NCORES)

    hp2_d = nc.dram_tensor("hp2", [128, 66 * 130], F32, kind="ExternalInput")
    h8_d = nc.dram_tensor("h8", [64, 66 * 130], BF16, kind="ExternalInput")
    w1a_d = nc.dram_tensor("w1a", [128, 3 * 128], F32, kind="ExternalInput")
    w1b_d = nc.dram_tensor("w1b", [64, 3 * 128], F32, kind="ExternalInput")
    b1_d = nc.dram_tensor("b1c", [128, 1], F32, kind="ExternalInput")
    w2t_d = nc.dram_tensor("w2t", [128, 36], F32, kind="ExternalInput")
    b2_d = nc.dram_tensor("b2c", [36, 1], F32, kind="ExternalInput")
    sel_d = nc.dram_tensor("sel", [36, 4], F32, kind="ExternalInput")
    idf_d = nc.dram_tensor("idf", [128, 128], F32, kind="ExternalInput")
    idb_d = nc.dram_tensor("idb", [128, 128], BF16, kind="ExternalInput")
    out_d = nc.dram_tensor("out", [64, H, 2 * W], F32, kind="ExternalOutput")

    NA = 4 * YB          # acc free per c: (ff, y_local)
    n = YB * 64          # per-(tap,block) product elements per partition

    with tile.TileContext(nc) as tc:
        with (
            tc.tile_pool(name="const", bufs=1) as cpool,
            tc.tile_pool(name="ring", bufs=2) as ring,
            tc.tile_pool(name="mchunk", bufs=3) as mpool,
            tc.tile_pool(name="ws1", bufs=2) as wp2,
            tc.tile_pool(name="ws2", bufs=1) as wp1,
            tc.tile_pool(name="orow", bufs=3) as opool,
            tc.tile_pool(name="ps1", bufs=2, space=bass.MemorySpace.PSUM) as pp1,
            tc.tile_pool(name="ps2", bufs=2, space=bass.MemorySpace.PSUM) as pp2,
            tc.tile_pool(name="psz", bufs=1, space=bass.MemorySpace.PSUM) as ppz,
            tc.tile_pool(name="pst", bufs=1, space=bass.MemorySpace.PSUM) as ppt,
            tc.tile_pool(name="psh", bufs=1, space=bass.MemorySpace.PSUM) as pph,
            tc.tile_pool(name="pso", bufs=1, space=bass.MemorySpace.PSUM) as ppo,
        ):
            # ---- constants ----
            w1a = cpool.tile([128, 3 * 128], F32)
            w1b = cpool.tile([64, 3 * 128], F32)
            b1 = cpool.tile([128, 1], F32)
            w2t = cpool.tile([128, 36], F32)
            b2 = cpool.tile([36, 1], F32)
            sel = cpool.tile([36, 4], F32)
            idf = cpool.tile([128, 128], F32)
            idb = cpool.tile([128, 128], BF16)
            nc.sync.dma_start(w1a[:], w1a_d[:])
            nc.sync.dma_start(w1b[:], w1b_d[:])
            nc.sync.dma_start(b1[:], b1_d[:])
            nc.sync.dma_start(w2t[:], w2t_d[:])
            nc.sync.dma_start(b2[:], b2_d[:])
            nc.sync.dma_start(sel[:], sel_d[:])
            nc.sync.dma_start(idf[:], idf_d[:])
            nc.sync.dma_start(idb[:], idb_d[:])

            for yb in range(NYB):
                r0 = yb * YB  # first coarse row of this block
                hp2b = ring.tile([128, 18 * 130], F32, tag="hp2b")
                h8b = ring.tile([64, 18 * 130], BF16, tag="h8b")
                nc.sync.dma_start(hp2b[:], hp2_d[:, r0 * 130:(r0 + 18) * 130])
                nc.sync.dma_start(h8b[:], h8_d[:, r0 * 130:(r0 + 18) * 130])

                # ---- conv1 -> relu -> conv2 -> exp -> Z -> recip (4 chunks) ----
                eb = ring.tile([36, 4 * 512], F32, tag="eb")
                rzb = ring.tile([4, 4 * 512], F32, tag="rzb")
                for ic in range(4):
                    ps1 = pp1.tile([128, 512], F32)
                    for dy in range(3):
                        rhs = ap_of(hp2b, (4 * ic + dy) * 130,
                                    [[18 * 130, 128], [130, 4], [1, 128]])
                        nc.tensor.matmul(ps1[:], w1a[:, dy * 128:(dy + 1) * 128], rhs,
                                         start=(dy == 0), stop=False)
                    for dy in range(3):
                        rhs = ap_of(hp2b, (4 * ic + dy) * 130 + 2,
                                    [[18 * 130, 64], [130, 4], [1, 128]])
                        nc.tensor.matmul(ps1[:], w1b[:, dy * 128:(dy + 1) * 128], rhs,
                                         start=False, stop=(dy == 2))
                    m = mpool.tile([128, 512], F32)
                    nc.scalar.activation(m[:], ps1[:], AF.Relu, bias=b1[:], scale=1.0)
                    ps2 = pp2.tile([40, 512], F32)
                    nc.tensor.matmul(ps2[0:36, :], w2t[:], m[:])
                    nc.scalar.activation(eb[:, ic * 512:(ic + 1) * 512],
                                         ps2[0:36, :], AF.Exp, bias=b2[:], scale=1.0)
                    psz = ppz.tile([4, 512], F32)
                    nc.tensor.matmul(psz[:], sel[:], eb[0:36, ic * 512:(ic + 1) * 512])
                    nc.vector.reciprocal(rzb[:, ic * 512:(ic + 1) * 512], psz[:])

                # ---- h transposes (bf16), batched PSUM->SBUF copies ----
                hTb = ring.tile([128, 3 * 18 * 64], BF16, tag="hTb")
                for dx in range(3):
                    for j in range(3):   # 3 batches of 6 rows
                        psh = pph.tile([128, 6 * 64], BF16)
                        for r in range(6):
                            yp = j * 6 + r
                            nc.tensor.transpose(
                                psh[:, r * 64:(r + 1) * 64],
                                ap_of(h8b, yp * 130 + dx, [[18 * 130, 64], [1, 128]]),
                                idb[0:64, 0:64])
                        nc.scalar.copy(
                            hTb[:, (dx * 18 + j * 6) * 64:(dx * 18 + j * 6 + 6) * 64],
                            psh[:])

                # ---- e/rz transposes, batched ----
                eTb = ring.tile([128, YB * 40], F32, tag="eTb")
                for j in range(4):       # 4 batches of 4 rows
                    pst = ppt.tile([128, 4 * 40], F32)
                    for r in range(4):
                        yl = j * 4 + r
                        nc.tensor.transpose(pst[:, r * 40:r * 40 + 36],
                                            eb[:, yl * 128:(yl + 1) * 128],
                                            idf[0:36, 0:36])
                        nc.tensor.transpose(pst[:, r * 40 + 36:r * 40 + 40],
                                            rzb[:, yl * 128:(yl + 1) * 128],
                                            idf[0:4, 0:4])
                    nc.scalar.copy(eTb[:, j * 160:(j + 1) * 160], pst[:])

                # ---- normalized mask, transposed+duplicated (bf16) ----
                nmb = ring.tile([128, YB * 72], BF16, tag="nmb")
                for ff in range(NFF):
                    out_ap = ap_of(nmb, ff * 18, [[YB * 72, 128], [72, YB], [2, 9], [1, 2]])
                    in0 = ap_of(eTb, ff * 9, [[YB * 40, 128], [40, YB], [1, 9], [0, 2]])
                    in1 = ap_of(eTb, 36 + ff, [[YB * 40, 128], [40, YB], [0, 9], [0, 2]])
                    nc.vector.tensor_tensor(out_ap, in0, in1, ALU.mult)

                # ---- weighted tap sum (DVE, bf16) ----
                acc = ring.tile([128, 64 * NA], F32, tag="acc")  # (c, ff, yl)
                for ff in range(NFF):
                    prod = wp2.tile([128, NT * n], BF16, tag="prod")
                    for dy in range(3):
                        for dx in range(3):
                            t = dy * 3 + dx
                            in0 = ap_of(hTb, (dx * 18 + dy) * 64,
                                        [[3 * 18 * 64, 128], [64, YB], [2, 32], [1, 2]])
                            in1 = ap_of(nmb, (ff * 9 + t) * 2,
                                        [[YB * 72, 128], [72, YB], [0, 32], [1, 2]])
                            po = ap_of(prod, t * n,
                                       [[NT * n, 128], [64, YB], [2, 32], [1, 2]])
                            nc.vector.tensor_tensor(po, in0, in1, ALU.mult)
                    tA = wp2.tile([128, 4 * n], BF16, tag="tA")
                    tB = wp1.tile([128, 2 * n], BF16, tag="tB")
                    tC = wp1.tile([128, n], BF16, tag="tC")
                    nc.vector.tensor_add(tA[:], prod[:, 0:4 * n], prod[:, 4 * n:8 * n])
                    nc.vector.tensor_add(tB[:], tA[:, 0:2 * n], tA[:, 2 * n:4 * n])
                    nc.vector.tensor_add(tC[:], tB[:, 0:n], tB[:, n:2 * n])
                    acc_ap = ap_of(acc, ff * YB, [[64 * NA, 128], [1, YB], [NA, 64]])
                    tC_ap = ap_of(tC, 0, [[n, 128], [64, YB], [1, 64]])
                    p8_ap = ap_of(prod, 8 * n, [[NT * n, 128], [64, YB], [1, 64]])
                    nc.vector.tensor_tensor(acc_ap, tC_ap, p8_ap, ALU.add)

                # ---- pixel shuffle out ----
                for yl in range(YB):
                    y = yb * YB + yl
                    orow = opool.tile([128, 256], F32)
                    for fx in range(2):
                        pso = ppo.tile([128, 128], F32)
                        in_ap = ap_of(acc, fx * YB + yl,
                                      [[64 * NA, 128], [NA, 64], [2 * YB, 2]])
                        nc.tensor.transpose(pso[:], in_ap, idf[:])
                        o_ap = ap_of(orow, fx, [[256, 128], [2, 128]])
                        nc.scalar.copy(o_ap, pso[:])
                    nc.sync.dma_start(out_d[:, 2 * y:2 * y + 2, :], orow[:])

    nc.compile()
    return nc


def prep_shared(W1, b1, W2, b2):
    W1 = np.asarray(W1, np.float32)
    b1 = np.asarray(b1, np.float32)
    W2 = np.asarray(W2, np.float32).reshape(36, 128)
    b2 = np.asarray(b2, np.float32)

    w1a = np.zeros((128, 3 * 128), np.float32)
    w1b = np.zeros((64, 3 * 128), np.float32)
    for dy in range(3):
        w1a[0:64, dy * 128:(dy + 1) * 128] = W1[:, :, dy, 0].T
        w1a[64:128, dy * 128:(dy + 1) * 128] = W1[:, :, dy, 1].T
        w1b[:, dy * 128:(dy + 1) * 128] = W1[:, :, dy, 2].T

    o_of_mp = np.array([t * 4 + ff for ff in range(4) for t in range(9)])
    w2t = np.ascontiguousarray((0.25 * W2[o_of_mp, :]).T)
    b2c = np.ascontiguousarray((0.25 * b2[o_of_mp]).reshape(36, 1))

    sel = np.zeros((36, 4), np.float32)
    for k in range(36):
        sel[k, k // 9] = 1.0
    idf = np.eye(128, dtype=np.float32)
    return {
        "w1a": w1a, "w1b": w1b, "b1c": b1.reshape(128, 1).astype(np.float32),
        "w2t": w2t.astype(np.float32), "b2c": b2c, "sel": sel, "idf": idf,
        "idb": np.eye(128, dtype=ml_dtypes.bfloat16),
    }


def kernel(h, W1, b1, W2, b2, _trace=False):
    h = np.asarray(h, np.float32)
    shared = prep_shared(W1, b1, W2, b2)

    hp = np.pad(h, ((0, 0), (0, 0), (1, 1), (1, 1)))  # [B, C, 130, 130]
    in_maps = []
    for core in range(NCORES):
        b, half = core // 2, core % 2
        y0 = half * HL
        win = hp[b, :, y0:y0 + 66, :]  # [64, 66, 130]
        hp2 = np.zeros((128, 66, 130), np.float32)
        hp2[0:64] = win
        hp2[64:128, :, 0:129] = win[:, :, 1:130]
        h8 = (8.0 * win).astype(np.float32)
        m = dict(shared)
        m["hp2"] = hp2.reshape(128, -1)
        m["h8"] = np.ascontiguousarray(h8.reshape(64, -1)).astype(ml_dtypes.bfloat16)
        in_maps.append(m)

    if "nc" not in _cached:
        _cached["nc"] = build_nc()
    res = run_bass_kernel_spmd(_cached["nc"], in_maps, core_ids=list(range(NCORES)),
                               trace=_trace)

    out = np.zeros((B, C, 2 * H, 2 * W), np.float32)
    for core in range(NCORES):
        b, half = core // 2, core % 2
        out[b, :, half * 128:(half + 1) * 128, :] = res.results[core]["out"]
    if _trace:
        return out, res
    return out



# revision 11
# speedup vs baseline: 2.1128x; 2.1128x over previous
import sys

sys.path.insert(0, "/opt/trn_rl_repo")

import numpy as np
import ml_dtypes

import concourse.bacc as bacc
import concourse.bass as bass
import concourse.mybir as mybir
import concourse.tile as tile
from concourse.bass_utils import run_bass_kernel_spmd

F32 = mybir.dt.float32
BF16 = mybir.dt.bfloat16
AF = mybir.ActivationFunctionType
ALU = mybir.AluOpType
AX = mybir.AxisListType

# Problem constants (hardcoded per harness contract).
B, C, H, W = 4, 64, 128, 128
NT = 9          # 3x3 taps
NFF = 4         # factor*factor subpixels
NCORES = 8
HL = H // 2     # 64 coarse rows per core
NYB = 4         # y-blocks
YB = HL // NYB  # 16 rows per block
WR = 18         # window rows per block (YB + 2 halo)
N = YB * 64     # elems per (tap, ff) product slab per partition

_cached = {}


def ap_of(t, off, dims):
    base = t[:]
    return bass.AP(base.tensor, base.offset + off, dims)


def build_nc():
    nc = bacc.Bacc("TRN2", target_bir_lowering=False, debug=False, num_devices=NCORES)

    hp2_d = nc.dram_tensor("hp2", [128, 66 * 130], BF16, kind="ExternalInput")
    hT3_d = nc.dram_tensor("hT3", [128, 3 * 66 * 64], BF16, kind="ExternalInput")
    w1a_d = nc.dram_tensor("w1a", [128, 3 * 128], BF16, kind="ExternalInput")
    w1b_d = nc.dram_tensor("w1b", [64, 3 * 128], BF16, kind="ExternalInput")
    b1_d = nc.dram_tensor("b1c", [128, 1], F32, kind="ExternalInput")
    w2t_d = nc.dram_tensor("w2t", [128, 36], BF16, kind="ExternalInput")
    b2_d = nc.dram_tensor("b2c", [36, 1], F32, kind="ExternalInput")
    idb_d = nc.dram_tensor("idb", [128, 128], BF16, kind="ExternalInput")
    out_d = nc.dram_tensor("out", [64, H, 2 * W], BF16, kind="ExternalOutput")

    with tile.TileContext(nc) as tc:
        with (
            tc.tile_pool(name="const", bufs=1) as cpool,
            tc.tile_pool(name="ring", bufs=2) as ring,
            tc.tile_pool(name="mchunk", bufs=2) as mpool,
            tc.tile_pool(name="masks", bufs=2) as kpool,
            tc.tile_pool(name="prodp", bufs=2) as ppool,
            tc.tile_pool(name="accp", bufs=2) as apool,
            tc.tile_pool(name="orow", bufs=3) as opool,
            tc.tile_pool(name="ps1", bufs=2, space=bass.MemorySpace.PSUM) as pp1,
            tc.tile_pool(name="ps2", bufs=1, space=bass.MemorySpace.PSUM) as pp2,
            tc.tile_pool(name="pst", bufs=1, space=bass.MemorySpace.PSUM) as ppt,
            tc.tile_pool(name="psa", bufs=2, space=bass.MemorySpace.PSUM) as ppa,
            tc.tile_pool(name="pso", bufs=2, space=bass.MemorySpace.PSUM) as ppo,
        ):
            # ---- constants ----
            w1a = cpool.tile([128, 3 * 128], BF16)
            w1b = cpool.tile([64, 3 * 128], BF16)
            b1 = cpool.tile([128, 1], F32)
            w2t = cpool.tile([128, 36], BF16)
            b2 = cpool.tile([36, 1], F32)
            idb = cpool.tile([128, 128], BF16)
            nc.sync.dma_start(w1a[:], w1a_d[:])
            nc.sync.dma_start(w1b[:], w1b_d[:])
            nc.sync.dma_start(b1[:], b1_d[:])
            nc.sync.dma_start(w2t[:], w2t_d[:])
            nc.sync.dma_start(b2[:], b2_d[:])
            nc.sync.dma_start(idb[:], idb_d[:])

            def phase1(yb):
                """DMA + conv1 + relu + conv2 + exp + mask transpose + norm +
                DVE products for block yb. Returns (prod, state)."""
                r0 = yb * YB
                hp2b = ring.tile([128, WR * 130], BF16, tag="hp2b")
                hT3b = ring.tile([128, 3 * WR * 64], BF16, tag="hT3b")
                nc.sync.dma_start(hp2b[:], hp2_d[:, r0 * 130:(r0 + WR) * 130])
                nc.sync.dma_start(
                    hT3b[:],
                    ap_of(hT3_d, r0 * 64, [[3 * 66 * 64, 128], [66 * 64, 3], [1, WR * 64]]))

                # conv1 -> relu -> conv2 -> exp, in 4 chunks of 4 rows
                m1 = mpool.tile([128, 4 * 512], BF16, tag="m1")
                eb = mpool.tile([36, 4 * 512], BF16, tag="eb")
                for ic in range(4):
                    ps1 = pp1.tile([128, 512], F32)
                    for dy in range(3):
                        rhs = ap_of(hp2b, (4 * ic + dy) * 130,
                                    [[WR * 130, 128], [130, 4], [1, 128]])
                        nc.tensor.matmul(ps1[:], w1a[:, dy * 128:(dy + 1) * 128], rhs,
                                         start=(dy == 0), stop=False)
                    for dy in range(3):
                        rhs = ap_of(hp2b, (4 * ic + dy) * 130 + 2,
                                    [[WR * 130, 64], [130, 4], [1, 128]])
                        nc.tensor.matmul(ps1[:], w1b[:, dy * 128:(dy + 1) * 128], rhs,
                                         start=False, stop=(dy == 2))
                    nc.scalar.activation(m1[:, ic * 512:(ic + 1) * 512], ps1[:],
                                         AF.Relu, bias=b1[:], scale=1.0)
                    ps2 = pp2.tile([36, 512], F32)
                    nc.tensor.matmul(ps2[:], w2t[:], m1[:, ic * 512:(ic + 1) * 512])
                    nc.scalar.activation(eb[:, ic * 512:(ic + 1) * 512], ps2[:],
                                         AF.Exp, bias=b2[:], scale=1.0)

                # transpose masks: eb [36, (y,x)] -> eT [x, (y, 36)]
                pst = ppt.tile([128, YB * 36], BF16)
                for yl in range(YB):
                    nc.tensor.transpose(pst[:, yl * 36:(yl + 1) * 36],
                                        eb[:, yl * 128:(yl + 1) * 128],
                                        idb[0:36, 0:36])
                eT = kpool.tile([128, YB * 36], BF16, tag="eT")
                nc.scalar.copy(eT[:], pst[:])

                # Z = sum over taps (t innermost, stride 1), then 1/Z
                zb = kpool.tile([128, YB * 4], F32, tag="zb")
                rz = kpool.tile([128, YB * 4], F32, tag="rz")
                z_in = ap_of(eT, 0, [[YB * 36, 128], [36, YB], [9, 4], [1, 9]])
                z_out = ap_of(zb, 0, [[YB * 4, 128], [4, YB], [1, 4]])
                nc.vector.tensor_reduce(z_out, z_in, AX.X, ALU.add)
                nc.vector.reciprocal(rz[:], zb[:])

                # normalized masks duplicated in pairs: nm[x, (y, k, 2)]
                nm = kpool.tile([128, YB * 72], BF16, tag="nm")
                for ff in range(NFF):
                    o_ap = ap_of(nm, ff * 18, [[YB * 72, 128], [72, YB], [2, 9], [1, 2]])
                    i0 = ap_of(eT, ff * 9, [[YB * 36, 128], [36, YB], [1, 9], [0, 2]])
                    i1 = ap_of(rz, ff, [[YB * 4, 128], [4, YB], [0, 9], [0, 2]])
                    nc.vector.tensor_tensor(o_ap, i0, i1, ALU.mult)

                return hT3b, nm

            def phase2(yb, state):
                """DVE products + PE tap-sum + evict + output + DMA for block yb."""
                hT3b, nm = state
                sA = apool.tile([128, NFF * N], BF16, tag="sA")
                for ff in range(NFF):
                    # DVE products: prod[x, (t, y, c)]
                    prod = ppool.tile([128, NT * N], BF16, tag="prod")
                    for dy in range(3):
                        for dx in range(3):
                            t = dy * 3 + dx
                            i0 = ap_of(hT3b, dx * (WR * 64) + dy * 64,
                                       [[3 * WR * 64, 128], [64, YB], [2, 32], [1, 2]])
                            i1 = ap_of(nm, (ff * 9 + t) * 2,
                                       [[YB * 72, 128], [72, YB], [0, 32], [1, 2]])
                            po = ap_of(prod, t * N,
                                       [[NT * N, 128], [64, YB], [2, 32], [1, 2]])
                            nc.vector.tensor_tensor(po, i0, i1, ALU.mult)
                    # PE tap-sum via identity-matmul PSUM accumulation
                    fy, fx = ff // 2, ff % 2
                    for hf in range(2):
                        psA = ppa.tile([128, 512], F32)
                        for t in range(NT):
                            rhs = prod[:, t * N + hf * 512:t * N + hf * 512 + 512]
                            nc.tensor.matmul(psA[:], idb[:], rhs,
                                             start=(t == 0), stop=(t == NT - 1))
                        # evict interleaved: sA[x, (fx, y, c, fy)]
                        o_ap = ap_of(sA, fx * 2 * N + hf * 8 * 128 + fy,
                                     [[NFF * N, 128], [128, 8], [2, 64]])
                        i_ap = ap_of(psA, 0, [[512, 128], [64, 8], [1, 64]])
                        nc.scalar.copy(o_ap, i_ap)

                # output: per 2-row group, 4 transposes -> pso -> DMA
                # sA layout: (fx, y, (c,fy))
                for g in range(YB // 2):
                    pso = ppo.tile([128, 512], BF16)
                    for yy in range(2):
                        yl = 2 * g + yy
                        for fx in range(2):
                            in_ap = ap_of(sA, fx * 2 * N + yl * 128,
                                          [[NFF * N, 128], [1, 128]])
                            o_ap = ap_of(pso, (yy * 2 + fx) * 128,
                                         [[512, 128], [1, 128]])
                            nc.tensor.transpose(o_ap, in_ap, idb[:])
                    orow = opool.tile([128, 512], BF16, tag="orow")
                    # interleave (yy, fx, x) -> (yy, 2x+fx)
                    i_ap = ap_of(pso, 0, [[512, 128], [256, 2], [128, 2], [1, 128]])
                    o_ap = ap_of(orow, 0, [[512, 128], [256, 2], [1, 2], [2, 128]])
                    nc.scalar.copy(o_ap, i_ap)
                    y0 = yb * YB + 2 * g
                    for yy in range(2):
                        dst = ap_of(out_d, (2 * (y0 + yy)) * 256,
                                    [[H * 256, 64], [256, 2], [1, 256]])
                        nc.sync.dma_start(dst, orow[:, yy * 256:(yy + 1) * 256])

            # software pipeline: phase2 lags one block behind phase1
            prev = phase1(0)
            for yb in range(1, NYB):
                cur = phase1(yb)
                phase2(yb - 1, prev)
                prev = cur
            phase2(NYB - 1, prev)

    nc.compile()
    return nc


def prep_shared(W1, b1, W2, b2):
    W1 = np.asarray(W1, np.float32)
    b1 = np.asarray(b1, np.float32)
    W2 = np.asarray(W2, np.float32).reshape(36, 128)
    b2 = np.asarray(b2, np.float32)

    w1a = np.zeros((128, 3 * 128), np.float32)
    w1b = np.zeros((64, 3 * 128), np.float32)
    for dy in range(3):
        w1a[0:64, dy * 128:(dy + 1) * 128] = W1[:, :, dy, 0].T
        w1a[64:128, dy * 128:(dy + 1) * 128] = W1[:, :, dy, 1].T
        w1b[:, dy * 128:(dy + 1) * 128] = W1[:, :, dy, 2].T

    # eb row k = (ff = k//9, t = k%9) <- original channel t*4+ff
    o_of_mp = np.array([t * 4 + ff for ff in range(4) for t in range(9)])
    w2t = np.ascontiguousarray((0.25 * W2[o_of_mp, :]).T)
    b2c = np.ascontiguousarray((0.25 * b2[o_of_mp]).reshape(36, 1))

    return {
        "w1a": w1a.astype(ml_dtypes.bfloat16),
        "w1b": w1b.astype(ml_dtypes.bfloat16),
        "b1c": b1.reshape(128, 1).astype(np.float32),
        "w2t": w2t.astype(ml_dtypes.bfloat16),
        "b2c": b2c.astype(np.float32),
        "idb": np.eye(128, dtype=ml_dtypes.bfloat16),
    }


def kernel(h, W1, b1, W2, b2, _trace=False):
    h = np.asarray(h, np.float32)
    shared = prep_shared(W1, b1, W2, b2)

    hp = np.pad(h, ((0, 0), (0, 0), (1, 1), (1, 1)))  # [B, C, 130, 130]
    in_maps = []
    for core in range(NCORES):
        b, half = core // 2, core % 2
        y0 = half * HL
        win = hp[b, :, y0:y0 + 66, :]  # [64, 66, 130] f32
        hp2 = np.zeros((128, 66, 130), np.float32)
        hp2[0:64] = win
        hp2[64:128, :, 0:129] = win[:, :, 1:130]
        w8 = 8.0 * win
        # hT3[x, dx, yy, c] = 8*win[c, yy, x+dx]
        hT3 = np.stack([w8[:, :, d:d + 128] for d in range(3)], axis=0)  # [3,64,66,128]
        hT3 = np.ascontiguousarray(hT3.transpose(3, 0, 2, 1))  # [128,3,66,64]
        m = dict(shared)
        m["hp2"] = hp2.reshape(128, -1).astype(ml_dtypes.bfloat16)
        m["hT3"] = hT3.reshape(128, -1).astype(ml_dtypes.bfloat16)
        in_maps.append(m)

    if "nc" not in _cached:
        _cached["nc"] = build_nc()
    res = run_bass_kernel_spmd(_cached["nc"], in_maps, core_ids=list(range(NCORES)),
                               trace=_trace)

    out = np.zeros((B, C, 2 * H, 2 * W), np.float32)
    for core in range(NCORES):
        b, half = core // 2, core % 2
        out[b, :, half * 128:(half + 1) * 128, :] = np.asarray(
            res.results[core]["out"], dtype=np.float32)
    if _trace:
        return out, res
    return out


# revision 12
# speedup vs baseline: 2.2990x; 1.0881x over previous
import sys

sys.path.insert(0, "/opt/trn_rl_repo")

import numpy as np
import ml_dtypes

import concourse.bacc as bacc
import concourse.bass as bass
import concourse.mybir as mybir
import concourse.tile as tile
from concourse.bass_utils import run_bass_kernel_spmd

F32 = mybir.dt.float32
BF16 = mybir.dt.bfloat16
AF = mybir.ActivationFunctionType
ALU = mybir.AluOpType
AX = mybir.AxisListType

# Problem constants (hardcoded per harness contract).
B, C, H, W = 4, 64, 128, 128
NT = 9          # 3x3 taps
NFF = 4         # factor*factor subpixels
NCORES = 8
HL = H // 2     # 64 coarse rows per core
NYB = 8         # y-blocks
YB = HL // NYB  # 8 rows per block
WR = YB + 2     # window rows per block (halo)
N = YB * 64     # elems per (tap, ff) product slab per partition

_cached = {}


def ap_of(t, off, dims):
    base = t[:]
    return bass.AP(base.tensor, base.offset + off, dims)


def build_nc():
    nc = bacc.Bacc("TRN2", target_bir_lowering=False, debug=False, num_devices=NCORES)

    hp2_d = nc.dram_tensor("hp2", [128, 66 * 130], BF16, kind="ExternalInput")
    hps_d = nc.dram_tensor("hps", [128, 66 * 130], BF16, kind="ExternalInput")
    hT3_d = nc.dram_tensor("hT3", [128, 3 * 66 * 64], BF16, kind="ExternalInput")
    w1p_d = nc.dram_tensor("w1p", [128, 4 * 128], BF16, kind="ExternalInput")
    w1s_d = nc.dram_tensor("w1s", [64, 128], BF16, kind="ExternalInput")
    b1_d = nc.dram_tensor("b1c", [128, 1], F32, kind="ExternalInput")
    w2t_d = nc.dram_tensor("w2t", [128, 36], BF16, kind="ExternalInput")
    b2_d = nc.dram_tensor("b2c", [36, 1], F32, kind="ExternalInput")
    idb_d = nc.dram_tensor("idb", [128, 128], BF16, kind="ExternalInput")
    out_d = nc.dram_tensor("out", [64, H, 2 * W], BF16, kind="ExternalOutput")

    with tile.TileContext(nc) as tc:
        with (
            tc.tile_pool(name="const", bufs=1) as cpool,
            tc.tile_pool(name="ring", bufs=2) as ring,
            tc.tile_pool(name="mchunk", bufs=2) as mpool,
            tc.tile_pool(name="masks", bufs=2) as kpool,
            tc.tile_pool(name="prodp", bufs=2) as ppool,
            tc.tile_pool(name="accp", bufs=2) as apool,
            tc.tile_pool(name="orow", bufs=3) as opool,
            tc.tile_pool(name="ps1", bufs=2, space=bass.MemorySpace.PSUM) as pp1,
            tc.tile_pool(name="ps2", bufs=1, space=bass.MemorySpace.PSUM) as pp2,
            tc.tile_pool(name="pst", bufs=1, space=bass.MemorySpace.PSUM) as ppt,
            tc.tile_pool(name="psa", bufs=2, space=bass.MemorySpace.PSUM) as ppa,
            tc.tile_pool(name="pso", bufs=2, space=bass.MemorySpace.PSUM) as ppo,
        ):
            # ---- constants ----
            w1p = cpool.tile([128, 4 * 128], BF16)
            w1s = cpool.tile([64, 128], BF16)
            b1 = cpool.tile([128, 1], F32)
            w2t = cpool.tile([128, 36], BF16)
            b2 = cpool.tile([36, 1], F32)
            idb = cpool.tile([128, 128], BF16)
            nc.sync.dma_start(w1p[:], w1p_d[:])
            nc.sync.dma_start(w1s[:], w1s_d[:])
            nc.sync.dma_start(b1[:], b1_d[:])
            nc.sync.dma_start(w2t[:], w2t_d[:])
            nc.sync.dma_start(b2[:], b2_d[:])
            nc.sync.dma_start(idb[:], idb_d[:])

            def dma_in(yb):
                r0 = yb * YB
                hp2b = ring.tile([128, WR * 130], BF16, tag="hp2b")
                hpsb = ring.tile([128, WR * 130], BF16, tag="hpsb")
                hT3b = ring.tile([128, 3 * WR * 64], BF16, tag="hT3b")
                nc.sync.dma_start(hp2b[:], hp2_d[:, r0 * 130:(r0 + WR) * 130])
                nc.sync.dma_start(hpsb[:], hps_d[:, r0 * 130:(r0 + WR) * 130])
                nc.sync.dma_start(
                    hT3b[:],
                    ap_of(hT3_d, r0 * 64, [[3 * 66 * 64, 128], [66 * 64, 3], [1, WR * 64]]))
                return hp2b, hpsb, hT3b

            def conv(yb, bufs):
                """conv1 -> relu -> conv2 -> exp, in 2 chunks of 4 rows."""
                hp2b, hpsb, hT3b = bufs
                m1 = mpool.tile([128, 2 * 512], BF16, tag="m1")
                eb = mpool.tile([36, 2 * 512], BF16, tag="eb")
                for ic in range(2):
                    r = 4 * ic
                    ps1 = pp1.tile([128, 512], F32)
                    # 5 paired-tap matmuls: (t0,t1)x(buf, row off, col off)
                    plan = [(hp2b, 0, 0), (hpsb, 0, 2), (hp2b, 1, 1), (hp2b, 2, 0)]
                    for k, (buf, dr, dc) in enumerate(plan):
                        rhs = ap_of(buf, (r + dr) * 130 + dc,
                                    [[WR * 130, 128], [130, 4], [1, 128]])
                        nc.tensor.matmul(ps1[:], w1p[:, k * 128:(k + 1) * 128], rhs,
                                         start=(k == 0), stop=False)
                    rhs = ap_of(hp2b, (r + 2) * 130 + 2,
                                [[WR * 130, 64], [130, 4], [1, 128]])
                    nc.tensor.matmul(ps1[:], w1s[:], rhs, start=False, stop=True)
                    nc.scalar.activation(m1[:, ic * 512:(ic + 1) * 512], ps1[:],
                                         AF.Relu, bias=b1[:], scale=1.0)
                    ps2 = pp2.tile([36, 512], F32)
                    nc.tensor.matmul(ps2[:], w2t[:], m1[:, ic * 512:(ic + 1) * 512])
                    nc.scalar.activation(eb[:, ic * 512:(ic + 1) * 512], ps2[:],
                                         AF.Exp, bias=b2[:], scale=1.0)
                return eb

            def masks(yb, eb):
                """transpose masks, Z, 1/Z, normalized dup masks."""
                pst = ppt.tile([128, YB * 36], BF16)
                for yl in range(YB):
                    nc.tensor.transpose(pst[:, yl * 36:(yl + 1) * 36],
                                        eb[:, yl * 128:(yl + 1) * 128],
                                        idb[0:36, 0:36])
                eT = kpool.tile([128, YB * 36], BF16, tag="eT")
                nc.scalar.copy(eT[:], pst[:])

                zb = kpool.tile([128, YB * 4], F32, tag="zb")
                rz = kpool.tile([128, YB * 4], F32, tag="rz")
                z_in = ap_of(eT, 0, [[YB * 36, 128], [36, YB], [9, 4], [1, 9]])
                z_out = ap_of(zb, 0, [[YB * 4, 128], [4, YB], [1, 4]])
                nc.vector.tensor_reduce(z_out, z_in, AX.X, ALU.add)
                nc.vector.reciprocal(rz[:], zb[:])

                nm = kpool.tile([128, YB * 72], BF16, tag="nm")
                for ff in range(NFF):
                    o_ap = ap_of(nm, ff * 18, [[YB * 72, 128], [72, YB], [2, 9], [1, 2]])
                    i0 = ap_of(eT, ff * 9, [[YB * 36, 128], [36, YB], [1, 9], [0, 2]])
                    i1 = ap_of(rz, ff, [[YB * 4, 128], [4, YB], [0, 9], [0, 2]])
                    nc.vector.tensor_tensor(o_ap, i0, i1, ALU.mult)
                return nm

            def units(yb, state):
                """DVE products + PE tap-sum + evict for block yb."""
                hT3b, nm = state
                sA = apool.tile([128, NFF * N], BF16, tag="sA")
                for ff in range(NFF):
                    prod = ppool.tile([128, NT * N], BF16, tag="prod")
                    for dy in range(3):
                        for dx in range(3):
                            t = dy * 3 + dx
                            i0 = ap_of(hT3b, dx * (WR * 64) + dy * 64,
                                       [[3 * WR * 64, 128], [64, YB], [2, 32], [1, 2]])
                            i1 = ap_of(nm, (ff * 9 + t) * 2,
                                       [[YB * 72, 128], [72, YB], [0, 32], [1, 2]])
                            po = ap_of(prod, t * N,
                                       [[NT * N, 128], [64, YB], [2, 32], [1, 2]])
                            nc.vector.tensor_tensor(po, i0, i1, ALU.mult)
                    fy, fx = ff // 2, ff % 2
                    psA = ppa.tile([128, N], F32)
                    for t in range(NT):
                        nc.tensor.matmul(psA[:], idb[:], prod[:, t * N:(t + 1) * N],
                                         start=(t == 0), stop=(t == NT - 1))
                    # evict interleaved: sA[x, (fx, y, (c,fy))]
                    o_ap = ap_of(sA, fx * 2 * N + fy,
                                 [[NFF * N, 128], [128, YB], [2, 64]])
                    i_ap = ap_of(psA, 0, [[N, 128], [64, YB], [1, 64]])
                    nc.scalar.copy(o_ap, i_ap)
                return sA

            def outT(yb, sA):
                """output transposes + interleave + DMA for block yb."""
                for g in range(YB // 2):
                    pso = ppo.tile([128, 512], BF16)
                    for yy in range(2):
                        yl = 2 * g + yy
                        for fx in range(2):
                            in_ap = ap_of(sA, fx * 2 * N + yl * 128,
                                          [[NFF * N, 128], [1, 128]])
                            o_ap = ap_of(pso, (yy * 2 + fx) * 128,
                                         [[512, 128], [1, 128]])
                            nc.tensor.transpose(o_ap, in_ap, idb[:])
                    orow = opool.tile([128, 512], BF16, tag="orow")
                    i_ap = ap_of(pso, 0, [[512, 128], [256, 2], [128, 2], [1, 128]])
                    o_ap = ap_of(orow, 0, [[512, 128], [256, 2], [1, 2], [2, 128]])
                    nc.scalar.copy(o_ap, i_ap)
                    y0 = yb * YB + 2 * g
                    for yy in range(2):
                        dst = ap_of(out_d, (2 * (y0 + yy)) * 256,
                                    [[H * 256, 64], [256, 2], [1, 256]])
                        nc.sync.dma_start(dst, orow[:, yy * 256:(yy + 1) * 256])

            # software pipeline, one block lag for the product/sum/output stage
            bufs = dma_in(0)
            eb = conv(0, bufs)
            prev = (bufs[2], masks(0, eb))
            prev_sA = None
            for yb in range(1, NYB):
                bufs = dma_in(yb)
                eb = conv(yb, bufs)
                sA = units(yb - 1, prev)
                nm = masks(yb, eb)
                if prev_sA is not None:
                    outT(yb - 2, prev_sA)
                prev = (bufs[2], nm)
                prev_sA = sA
            sA = units(NYB - 1, prev)
            outT(NYB - 2, prev_sA)
            outT(NYB - 1, sA)

    nc.compile()
    return nc


def prep_shared(W1, b1, W2, b2):
    W1 = np.asarray(W1, np.float32)
    b1 = np.asarray(b1, np.float32)
    W2 = np.asarray(W2, np.float32).reshape(36, 128)
    b2 = np.asarray(b2, np.float32)

    # paired-tap conv1 weights: pairs (t0 on parts 0:64, t1 on parts 64:128)
    # P1=(0,0)+(0,1) via hp2; P2=(0,2)+(1,0) via hps; P3=(1,1)+(1,2) via hp2;
    # P4=(2,0)+(2,1) via hp2; P5=(2,2) single via hp2 top half.
    pairs = [((0, 0), (0, 1)), ((0, 2), (1, 0)), ((1, 1), (1, 2)), ((2, 0), (2, 1))]
    w1p = np.zeros((128, 4 * 128), np.float32)
    for k, (ta, tb) in enumerate(pairs):
        w1p[0:64, k * 128:(k + 1) * 128] = W1[:, :, ta[0], ta[1]].T
        w1p[64:128, k * 128:(k + 1) * 128] = W1[:, :, tb[0], tb[1]].T
    w1s = np.ascontiguousarray(W1[:, :, 2, 2].T)

    # eb row k = (ff = k//9, t = k%9) <- original channel t*4+ff
    o_of_mp = np.array([t * 4 + ff for ff in range(4) for t in range(9)])
    w2t = np.ascontiguousarray((0.25 * W2[o_of_mp, :]).T)
    b2c = np.ascontiguousarray((0.25 * b2[o_of_mp]).reshape(36, 1))

    return {
        "w1p": w1p.astype(ml_dtypes.bfloat16),
        "w1s": w1s.astype(ml_dtypes.bfloat16),
        "b1c": b1.reshape(128, 1).astype(np.float32),
        "w2t": w2t.astype(ml_dtypes.bfloat16),
        "b2c": b2c.astype(np.float32),
        "idb": np.eye(128, dtype=ml_dtypes.bfloat16),
    }


def kernel(h, W1, b1, W2, b2, _trace=False):
    h = np.asarray(h, np.float32)
    shared = prep_shared(W1, b1, W2, b2)

    hp = np.pad(h, ((0, 0), (0, 0), (1, 1), (1, 1)))  # [B, C, 130, 130]
    in_maps = []
    for core in range(NCORES):
        b, half = core // 2, core % 2
        y0 = half * HL
        win = hp[b, :, y0:y0 + 66, :]  # [64, 66, 130] f32
        winf = win.reshape(64, -1)
        hp2 = np.zeros((128, 66 * 130), np.float32)
        hp2[0:64] = winf
        hp2[64:128, 0:66 * 130 - 1] = winf[:, 1:]
        hps = np.zeros((128, 66 * 130), np.float32)
        hps[0:64] = winf
        hps[64:128, 0:66 * 130 - 128] = winf[:, 128:]
        w8 = 8.0 * win
        # hT3[x, dx, yy, c] = 8*win[c, yy, x+dx]
        hT3 = np.stack([w8[:, :, d:d + 128] for d in range(3)], axis=0)  # [3,64,66,128]
        hT3 = np.ascontiguousarray(hT3.transpose(3, 0, 2, 1))  # [128,3,66,64]
        m = dict(shared)
        m["hp2"] = hp2.astype(ml_dtypes.bfloat16)
        m["hps"] = hps.astype(ml_dtypes.bfloat16)
        m["hT3"] = hT3.reshape(128, -1).astype(ml_dtypes.bfloat16)
        in_maps.append(m)

    if "nc" not in _cached:
        _cached["nc"] = build_nc()
    res = run_bass_kernel_spmd(_cached["nc"], in_maps, core_ids=list(range(NCORES)),
                               trace=_trace)

    out = np.zeros((B, C, 2 * H, 2 * W), np.float32)
    for core in range(NCORES):
        b, half = core // 2, core % 2
        out[b, :, half * 128:(half + 1) * 128, :] = np.asarray(
            res.results[core]["out"], dtype=np.float32)
    if _trace:
        return out, res
    return out
